# revision 13
# baseline (speedup 1.0000x reference)
"""Trainium2 Bass kernel for nn_CustomLoss: sum((predicted - target)**2) / 2.

Data-parallel across 8 NeuronCores: rows are sharded, each core streams its
128 MiB shard through SBUF and computes per-partition partial sums of
squared differences; the host sums the 8x128xNSEQ partials and halves.

Raw Bass (not Tile): the walrus codegen on this path allows only one sync
wait per compute instruction, so sync is explicit standalone wait_ge's.

Pipeline per core (2 double-buffered slots for the 30 full tiles, plus 4
dedicated single-use buffers for the tail chunks):
  SP ring   : pred DMAs (HWDGE queue 1)
  ACT ring  : targ DMAs (HWDGE queue 2, interleaved with squares)
  DVE       : diff = pred - targ (in place over pred)
  ACT       : square(diff) in place + per-partition accumulate -> acc[:, seq]

Because the tail chunks own their buffers, every DMA trigger is enqueued
long before the ring reaches it, so the two rings stream the full 128 MiB
back to back; the only exposed latency is ~5 us of preamble (runtime fixed
cost) and the last chunk's subtract+square (~4 us).

The Bass-init all-engine barrier is suppressed: its only purpose is
ordering the Pool const-AP memsets against consumers, and this kernel uses
an ACT-local memzero'd bias tile instead of the const APs.

Self-contained: hardcodes shapes from the problem spec; only depends on the
container's bass/concourse install at /opt/trn_rl_repo.
"""

import sys

if "/opt/trn_rl_repo" not in sys.path:
    sys.path.insert(0, "/opt/trn_rl_repo")

import numpy as np

N, D = 1048576, 128
NCORES = 8
ELEMS_PER_CORE = (N // NCORES) * D  # 16,777,216 fp32 = 64 MiB per tensor
P = 128                    # SBUF partitions
FTOT = ELEMS_PER_CORE // P  # 131072 fp32 per partition per tensor
FBIG = 4096                # full tile: 2 MiB per tensor per DMA
FSMALL = 2048              # tail chunks: 1 MiB per tensor per DMA
NFULL = 30
NCHUNK = 4
assert NFULL * FBIG + NCHUNK * FSMALL == FTOT
NSEQ = NFULL + NCHUNK

# Set by test harness to capture a HW profile; harness-default is plain run.
TRACE = False
LAST_EXEC_NS = None

_cached_nc = None


def _build():
    from contextlib import ExitStack

    from concourse import bass, mybir

    # Suppress the Bass-init all-engine barrier (see module docstring).
    orig_barrier = bass.Bass.all_engine_barrier
    bass.Bass.all_engine_barrier = lambda self, *a, **k: None
    try:
        nc = bass.Bass()
    finally:
        bass.Bass.all_engine_barrier = orig_barrier

    f32 = mybir.dt.float32
    pred_ext = nc.declare_dram_parameter("predicted", [P, FTOT], f32, isOutput=False)
    targ_ext = nc.declare_dram_parameter("target", [P, FTOT], f32, isOutput=False)
    out_ext = nc.declare_dram_parameter("partials", [P, NSEQ], f32, isOutput=True)

    ctx = ExitStack()
    # one sem per chunk DMA: several chunk DMAs are in flight at once, so a
    # shared counting sem would be unsound (the total can reach the target
    # while one transfer is still partial)
    cp_p = [ctx.enter_context(nc.semaphore(f"cp_p{j}")) for j in range(NCHUNK)]
    cp_t = [ctx.enter_context(nc.semaphore(f"cp_t{j}")) for j in range(NCHUNK)]
    pred_c = [
        ctx.enter_context(nc.sbuf_tensor(f"pred_c{j}", [P, FSMALL], f32))
        for j in range(NCHUNK)
    ]
    targ_c = [
        ctx.enter_context(nc.sbuf_tensor(f"targ_c{j}", [P, FSMALL], f32))
        for j in range(NCHUNK)
    ]

    with (
        ctx,
        nc.semaphore("psem_a") as psem_a,
        nc.semaphore("psem_b") as psem_b,
        nc.semaphore("tsem_a") as tsem_a,
        nc.semaphore("tsem_b") as tsem_b,
        nc.semaphore("dve_sem") as dve_sem,
        nc.semaphore("act_sem") as act_sem,
        nc.semaphore("out_sem") as out_sem,
        nc.sbuf_tensor("pred_a", [P, FBIG], f32) as pred_a,
        nc.sbuf_tensor("pred_b", [P, FBIG], f32) as pred_b,
        nc.sbuf_tensor("targ_a", [P, FBIG], f32) as targ_a,
        nc.sbuf_tensor("targ_b", [P, FBIG], f32) as targ_b,
        nc.sbuf_tensor("zbias", [P, 1], f32) as zbias,
        nc.sbuf_tensor("acc", [P, NSEQ], f32) as acc,
        nc.Block() as block,
    ):
        pred_t = [pred_a, pred_b]
        targ_t = [targ_a, targ_b]
        psem = [psem_a, psem_b]
        tsem = [tsem_a, tsem_b]

        def chunk_off(j):
            return NFULL * FBIG + j * FSMALL

        def pred_ap(seq):
            if seq < NFULL:
                return pred_t[seq % 2][:]
            return pred_c[seq - NFULL][:]

        def targ_ap(seq):
            if seq < NFULL:
                return targ_t[seq % 2][:]
            return targ_c[seq - NFULL][:]

        def targ_dma(eng, seq):
            if seq < NFULL:
                off, sem = seq * FBIG, tsem[seq % 2]
            else:
                off, sem = chunk_off(seq - NFULL), cp_t[seq - NFULL]
            w = FBIG if seq < NFULL else FSMALL
            eng.dma_start(
                out=targ_ap(seq), in_=targ_ext[:, off : off + w]
            ).then_inc(sem, 16)

        @block.sync
        def _(sync):
            for seq in range(NFULL):
                if seq >= 2:
                    # slot reused: the square of seq-2 (last tenant reader
                    # and in-place writer) must be done
                    sync.wait_ge(act_sem, seq - 1)
                off = seq * FBIG
                sync.dma_start(
                    out=pred_t[seq % 2][:], in_=pred_ext[:, off : off + FBIG]
                ).then_inc(psem[seq % 2], 16)
            for j in range(NCHUNK):  # dedicated buffers: no gating at all
                off = chunk_off(j)
                sync.dma_start(
                    out=pred_c[j][:], in_=pred_ext[:, off : off + FSMALL]
                ).then_inc(cp_p[j], 16)
            sync.wait_ge(act_sem, NSEQ)
            sync.dma_start(out=out_ext[:], in_=acc[:]).then_inc(out_sem, 16)
            sync.wait_ge(out_sem, 16)

        @block.vector
        def _(vector):
            for seq in range(NSEQ):
                if seq < NFULL:
                    vector.wait_ge(psem[seq % 2], 16 * (seq // 2 + 1))
                    vector.wait_ge(tsem[seq % 2], 16 * (seq // 2 + 1))
                else:
                    vector.wait_ge(cp_p[seq - NFULL], 16)
                    vector.wait_ge(cp_t[seq - NFULL], 16)
                vector.tensor_sub(
                    out=pred_ap(seq), in0=pred_ap(seq), in1=targ_ap(seq)
                ).then_inc(dve_sem, 1)

        @block.scalar
        def _(scalar):
            # zero bias for Square, owned by ACT itself (program order makes
            # it visible to every square; avoids the framework const APs and
            # therefore any dependence on the suppressed init barrier)
            scalar.memzero(zbias[:])
            # targ DMAs ride the ACT HWDGE ring, interleaved with the
            # squares; full-tile slot-reuse safety is ACT program order (the
            # square of the previous tenant precedes each trigger), chunk
            # buffers are single-use and need no gating.
            targ_dma(scalar, 0)
            targ_dma(scalar, 1)
            for seq in range(NSEQ):
                scalar.wait_ge(dve_sem, seq + 1)
                # square(diff) in place + row-sum. In-place is safe: the
                # next writer of this region is a pred DMA gated on act_sem
                # (cross-engine sem => writes drained), never a DMA
                # triggered by ACT itself right after.
                scalar.activation(
                    out=pred_ap(seq),
                    in_=pred_ap(seq),
                    func=mybir.ActivationFunctionType.Square,
                    bias=zbias[:],
                    accum_out=acc[:, seq : seq + 1],
                ).then_inc(act_sem, 1)
                nxt = seq + 2
                if nxt < NFULL:
                    targ_dma(scalar, nxt)
                elif nxt == NFULL:  # last full targ sent; queue all chunks
                    for j in range(NCHUNK):
                        targ_dma(scalar, NFULL + j)

    return nc


def kernel(predicted, target):
    global _cached_nc, LAST_EXEC_NS
    from concourse.bass_utils import run_bass_kernel_spmd

    if _cached_nc is None:
        _cached_nc = _build()
    nc = _cached_nc

    p = np.ascontiguousarray(np.asarray(predicted, dtype=np.float32)).reshape(
        NCORES, P, FTOT
    )
    t = np.ascontiguousarray(np.asarray(target, dtype=np.float32)).reshape(
        NCORES, P, FTOT
    )
    in_maps = [{"predicted": p[c], "target": t[c]} for c in range(NCORES)]
    res = run_bass_kernel_spmd(nc, in_maps, list(range(NCORES)), trace=TRACE)
    LAST_EXEC_NS = res.exec_time_ns
    total = sum(r["partials"].sum(dtype=np.float64) for r in res.results)
    return np.float32(total / 2.0)
